# revision 1
# baseline (speedup 1.0000x reference)
"""Trainium2 Bass kernel for LinearPotential (RBF potential evaluation).

out[n] = sum_m c_m * exp(-||x_n - a_m||^2 * w_m),  w_m = 0.5 / p_m^2

Strategy (data-parallel over the 8 NeuronCores, points sharded, anchors
replicated — no collectives):

  arg[n,m] = 2w(a.x) - w*x_sq - w*a_sq + ln|c|      (fold |c| into the exp)
           = sum_k  P[k,n] * R[k,m]                 (K-row contraction)

  - TensorE: the contraction is evaluated as a matmul with points on the
    output-partition axis and anchors on the free axis. Full fp32 matmul is
    4x slow and fp32r is only ~2^-12 accurate, so each fp32 factor is split
    into 3 bf16 components and each scalar product is expanded into 6
    partial-product rows (errors ~2^-26 relative) => K = 4*6 + 3 = 27 bf16
    rows, which still streams at 1 column/cycle.
  - ScalarE: exp() + free-dim accumulation in a single ACTIVATE.  Anchors
    are permuted so positive coefficients come first: one ACTIVATE+accum per
    sign block, result = pos_accum - neg_accum (exp is positive; the sign
    cannot be folded into the exponent).
  - VectorE: the final [128, 128] subtract.

Self-contained: hardcodes shapes for N=131072 points, M=2048 anchors.
"""

import numpy as np
import ml_dtypes

import concourse.tile as tile
from concourse import bacc, mybir
from concourse.bass_utils import run_bass_kernel_spmd

N_CORES = 8
N_POINTS = 131072
N_ANCH = 2048
N_LOC = N_POINTS // N_CORES  # 16384 points per core
P = 128                      # partition dim / points per tile
N_TILES = N_LOC // P         # 128 tiles per core
K_ROWS = 27                  # 4 products x 6 split rows + 3 const rows
MM_N = 512                   # matmul free-dim tile (one PSUM bank, fp32)

_BF16 = ml_dtypes.bfloat16

_program_cache: dict = {}

# test-harness hooks (no effect on grading: default off)
TRACE = False
LAST_RESULTS = None


def _split3(v: np.ndarray):
    """Split fp64 array into 3 bf16 components h+m+l ~ v (rel err ~2^-27)."""
    h = v.astype(_BF16)
    r = v - h.astype(np.float64)
    m = r.astype(_BF16)
    r2 = r - m.astype(np.float64)
    l = r2.astype(_BF16)
    return h, m, l


def _product_rows(u64: np.ndarray, v64: np.ndarray):
    """Rows for an accurate scalar product u*v via 6 bf16 partial products.

    Returns (point_rows, anchor_rows): lists of 6 bf16 vectors each such that
    sum_i point_rows[i] (x) anchor_rows[i] ~= u (x) v with ~2^-26 rel error.
    """
    uh, um, ul = _split3(u64)
    vh, vm, vl = _split3(v64)
    return [uh, uh, um, um, uh, ul], [vh, vm, vh, vm, vl, vh]


def _build_program(m_pos: int):
    """Build + compile the per-core Bass program (same on all 8 cores)."""
    nc = bacc.Bacc("TRN2", target_bir_lowering=False, debug=False,
                   num_devices=N_CORES)
    pm_d = nc.dram_tensor("pm", [K_ROWS, N_LOC], mybir.dt.bfloat16,
                          kind="ExternalInput").ap()
    r_d = nc.dram_tensor("r", [K_ROWS, N_ANCH], mybir.dt.bfloat16,
                         kind="ExternalInput").ap()
    out_d = nc.dram_tensor("out", [N_LOC], mybir.dt.float32,
                           kind="ExternalOutput").ap()

    exp_f = mybir.ActivationFunctionType.Exp
    with tile.TileContext(nc) as tc:
        with (
            tc.tile_pool(name="const", bufs=1) as cpool,
            tc.tile_pool(name="scratch", bufs=3) as spool,
            tc.tile_pool(name="psum", bufs=2, space="PSUM") as ppool,
        ):
            pm = cpool.tile([K_ROWS, N_LOC], mybir.dt.bfloat16)
            rr = cpool.tile([K_ROWS, N_ANCH], mybir.dt.bfloat16)
            nc.sync.dma_start(rr[:], r_d[:])
            # chunked point-matrix load so the first matmuls start early
            n_chunks = 16
            cw = N_LOC // n_chunks
            for c in range(n_chunks):
                nc.sync.dma_start(
                    pm[:, c * cw : (c + 1) * cw], pm_d[:, c * cw : (c + 1) * cw]
                )

            sall = cpool.tile([P, N_TILES], mybir.dt.float32)
            negs = cpool.tile([P, N_TILES], mybir.dt.float32)
            res = cpool.tile([P, N_TILES], mybir.dt.float32)
            if m_pos == N_ANCH:
                nc.vector.memset(negs[:], 0.0)

            for i in range(N_TILES):
                ps = ppool.tile([P, N_ANCH], mybir.dt.float32)
                lhsT = pm[:, P * i : P * (i + 1)]
                for j in range(N_ANCH // MM_N):
                    nc.tensor.matmul(
                        ps[:, MM_N * j : MM_N * (j + 1)],
                        lhsT=lhsT,
                        rhs=rr[:, MM_N * j : MM_N * (j + 1)],
                        start=True,
                        stop=True,
                    )
                # One Exp ACTIVATE over the full anchor range; the hardware
                # accumulator gives S_all = sum_m |c| e^arg. The elementwise
                # output lands in fp16 scratch, from which VectorE re-sums
                # just the negative-coefficient block: out = S_all - 2*S_neg.
                sc = spool.tile([P, N_ANCH], mybir.dt.float16)
                nc.scalar.activation(
                    sc[:], ps[:], exp_f, accum_out=sall[:, i : i + 1]
                )
                if m_pos < N_ANCH:
                    nc.vector.reduce_sum(
                        negs[:, i : i + 1], sc[:, m_pos:N_ANCH],
                        axis=mybir.AxisListType.X,
                    )
            nc.vector.scalar_tensor_tensor(
                res[:], negs[:], -2.0, sall[:],
                mybir.AluOpType.mult, mybir.AluOpType.add,
            )
            nc.sync.dma_start(out_d.rearrange("(p i) -> p i", i=N_TILES), res[:])
    nc.compile()
    return nc


def _prep_host(locations3d, anchor_locations3d, anchor_coeffs,
               anchor_parameters):
    """Build the 27-row point/anchor factor matrices (fp64 -> bf16 splits)."""
    x64 = locations3d.astype(np.float64)            # [N, 3]
    a64 = anchor_locations3d.astype(np.float64)     # [M, 3]
    c64 = anchor_coeffs.astype(np.float64)          # [M]
    p64 = anchor_parameters.astype(np.float64)      # [M]

    w = 0.5 / (p64 * p64)                           # [M]
    a_sq = (a64 * a64).sum(axis=1)                  # [M]
    x_sq = (x64 * x64).sum(axis=1)                  # [N]

    # permute anchors: positive coeffs first
    order = np.argsort(c64 <= 0, kind="stable")     # False(=pos) first
    m_pos = int((c64 > 0).sum())
    a64 = a64[order]
    c64 = c64[order]
    w = w[order]
    a_sq = a_sq[order]

    ln_c = np.log(np.maximum(np.abs(c64), 1e-300))
    ln_c = np.maximum(ln_c, -60.0)                  # exp(-60) ~ 9e-27 ~ 0

    # anchor-side factors F_t and point-side factors u_t:
    #   arg = sum_c x_c*(2 w a_c) + x_sq*(-w) + 1*(-w a_sq + ln|c|)
    point_factors = [x64[:, 0], x64[:, 1], x64[:, 2], x_sq]
    anchor_factors = [2.0 * w * a64[:, 0], 2.0 * w * a64[:, 1],
                      2.0 * w * a64[:, 2], -w]
    const_anchor = -w * a_sq + ln_c

    p_rows, r_rows = [], []
    for u, v in zip(point_factors, anchor_factors):
        pr, rr = _product_rows(u, v)
        p_rows.extend(pr)
        r_rows.extend(rr)
    ch, cm, cl = _split3(const_anchor)
    ones = np.ones(x_sq.shape[0], dtype=_BF16)
    p_rows.extend([ones, ones, ones])
    r_rows.extend([ch, cm, cl])

    P27 = np.stack(p_rows).astype(_BF16)            # [27, N]
    R27 = np.stack(r_rows).astype(_BF16)            # [27, M]
    return P27, R27, m_pos


def kernel(locations3d, anchor_locations3d, anchor_coeffs, anchor_parameters):
    assert locations3d.shape == (N_POINTS, 3)
    assert anchor_locations3d.shape == (N_ANCH, 3)

    P27, R27, m_pos = _prep_host(
        locations3d, anchor_locations3d, anchor_coeffs, anchor_parameters
    )

    nc = _program_cache.get(m_pos)
    if nc is None:
        nc = _build_program(m_pos)
        _program_cache[m_pos] = nc

    in_maps = []
    for c in range(N_CORES):
        shard = P27[:, c * N_LOC : (c + 1) * N_LOC]
        # reorder columns so tile i column p holds local point 128p + i:
        # the accum layout then DMAs out contiguously per partition.
        shard = np.ascontiguousarray(
            shard.reshape(K_ROWS, N_TILES, P).transpose(0, 2, 1)
            .reshape(K_ROWS, N_LOC)
        )
        in_maps.append({"pm": shard, "r": R27})

    res = run_bass_kernel_spmd(
        nc, in_maps, core_ids=list(range(N_CORES)), trace=TRACE
    )
    global LAST_RESULTS
    LAST_RESULTS = res
    out = np.concatenate([res.results[c]["out"] for c in range(N_CORES)])
    return out.astype(np.float32)



# revision 5
# speedup vs baseline: 1.7839x; 1.7839x over previous
"""Trainium2 Bass kernel for LinearPotential (RBF potential evaluation).

out[n] = sum_m c_m * exp(-||x_n - a_m||^2 * w_m),  w_m = 0.5 / p_m^2

Strategy: the ScalarE exp ACTIVATE (1 elem/cycle/lane @ 1.2 GHz) is the hard
bottleneck for the dense [N, M] evaluation, so the kernel drops pairs that
cannot contribute: most anchors are narrow (w up to ~50) and their Gaussian
reaches only a small neighborhood.

  - Host: recursively median-split the points into 1024 spatially tight
    tiles of 128.  For each tile keep only anchors with
    w*(max(0, |center-a|-radius))^2 - ln|c| < T  (a conservative bound on
    the best-case exponent for any point in the tile; dropped terms are
    each < e^-T).  Mean kept ~= 48% of anchors.
  - The 8 cores run ONE compiled program (SPMD), so per-slot trip counts
    are shared: tiles are sorted by kept-anchor count and dealt to cores in
    groups of 8 consecutive tiles -> per-slot padding waste is tiny.
  - Device per slot: matmul (K=14 bf16 factor rows, split products for
    ~2^-17 arg accuracy) -> PSUM [128, S] -> ScalarE Exp with free-dim
    accumulation (S_all) + fp16 elementwise scratch -> VectorE re-sum of
    the negative-coefficient prefix (S_neg) -> out = S_all - 2*S_neg.

Self-contained: hardcodes shapes for N=131072 points, M=2048 anchors.
"""

import numpy as np
import ml_dtypes

import concourse.tile as tile
from concourse import bacc, mybir
from concourse.bass_utils import run_bass_kernel_spmd

N_CORES = 8
N_POINTS = 131072
N_ANCH = 2048
N_LOC = N_POINTS // N_CORES  # 16384 points per core
P = 128                      # partition dim / points per tile
N_SLOTS = N_LOC // P         # 128 program slots per core
K_ROWS = 14                  # 4 products x 3 split rows + 2 const rows
MM_N = 512                   # matmul free-dim tile (one PSUM bank, fp32)
THRESH = 6.0                 # keep anchors with w*gap^2 - ln|c| < THRESH
DMA_GROUP = 4                # slots per rt DMA

_BF16 = ml_dtypes.bfloat16

_program_cache: dict = {}

# test-harness hooks (no effect on grading: default off)
TRACE = False
LAST_RESULTS = None


def _split2(v: np.ndarray):
    """Split fp64 array into 2 bf16 components h+m ~ v (rel err ~2^-17)."""
    h = v.astype(_BF16)
    m = (v - h.astype(np.float64)).astype(_BF16)
    return h, m


def _median_split_tiles(x: np.ndarray):
    """Recursively split N points into N/128 tiles of exactly 128 points
    by median cuts along the widest dimension. Returns [n_tiles, 128]
    int64 index array."""
    n = x.shape[0]
    idx = np.arange(n)
    groups = [idx]
    while groups[0].shape[0] > P:
        nxt = []
        for g in groups:
            pts = x[g]
            dim = int(np.argmax(pts.max(0) - pts.min(0)))
            half = g.shape[0] // 2
            part = np.argpartition(pts[:, dim], half)
            nxt.append(g[part[:half]])
            nxt.append(g[part[half:]])
        groups = nxt
    return np.stack(groups)


def _build_program(s_slot, s_neg_slot, rt_total):
    """Build + compile the per-core Bass program (same on all 8 cores)."""
    nc = bacc.Bacc("TRN2", target_bir_lowering=False, debug=False,
                   num_devices=N_CORES)
    pm_d = nc.dram_tensor("pm", [K_ROWS, N_LOC], mybir.dt.bfloat16,
                          kind="ExternalInput").ap()
    rt_d = nc.dram_tensor("rt", [K_ROWS, rt_total], mybir.dt.bfloat16,
                          kind="ExternalInput").ap()
    out_d = nc.dram_tensor("out", [N_LOC], mybir.dt.float32,
                           kind="ExternalOutput").ap()

    rt_off = np.concatenate([[0], np.cumsum(s_slot)])
    exp_f = mybir.ActivationFunctionType.Exp
    with tile.TileContext(nc) as tc:
        with (
            tc.tile_pool(name="const", bufs=1) as cpool,
            tc.tile_pool(name="rtp", bufs=3) as rtpool,
            tc.tile_pool(name="scp", bufs=3) as spool,
            tc.tile_pool(name="psum", bufs=2, space="PSUM") as ppool,
        ):
            pm = cpool.tile([K_ROWS, N_LOC], mybir.dt.bfloat16)
            sall = cpool.tile([P, N_SLOTS], mybir.dt.float32)
            negs = cpool.tile([P, N_SLOTS], mybir.dt.float32)
            res = cpool.tile([P, N_SLOTS], mybir.dt.float32)

            # interleave point-matrix chunk loads with rt group loads so
            # the first matmuls start early (all on the same SP queue)
            n_chunks = 16
            cw = N_LOC // n_chunks
            rt_bufs = {}
            for g in range(0, N_SLOTS, DMA_GROUP):
                c = g // DMA_GROUP
                if c < n_chunks:
                    nc.sync.dma_start(
                        pm[:, c * cw : (c + 1) * cw],
                        pm_d[:, c * cw : (c + 1) * cw],
                    )
                lo = int(rt_off[g])
                hi = int(rt_off[min(g + DMA_GROUP, N_SLOTS)])
                rt = rtpool.tile([K_ROWS, hi - lo], mybir.dt.bfloat16)
                nc.sync.dma_start(rt[:], rt_d[:, lo:hi])
                for k in range(g, min(g + DMA_GROUP, N_SLOTS)):
                    rt_bufs[k] = (rt, int(rt_off[k]) - lo)

            for k in range(N_SLOTS):
                S = int(s_slot[k])
                Sn = int(s_neg_slot[k])
                rt, base = rt_bufs[k]
                ps = ppool.tile([P, 2048], mybir.dt.float32)
                lhsT = pm[:, P * k : P * (k + 1)]
                for j in range(0, S, MM_N):
                    L = min(MM_N, S - j)
                    nc.tensor.matmul(
                        ps[:, j : j + L],
                        lhsT=lhsT,
                        rhs=rt[:, base + j : base + j + L],
                        start=True,
                        stop=True,
                    )
                # One Exp ACTIVATE over the kept anchors; the hardware
                # accumulator gives S_all = sum_m |c| e^arg. The elementwise
                # output lands in fp16 scratch, from which VectorE re-sums
                # the negative-coefficient prefix: out = S_all - 2*S_neg.
                sc = spool.tile([P, 2048], mybir.dt.float16)
                nc.scalar.activation(
                    sc[:, :S], ps[:, :S], exp_f, accum_out=sall[:, k : k + 1]
                )
                nc.vector.reduce_sum(
                    negs[:, k : k + 1], sc[:, :Sn], axis=mybir.AxisListType.X
                )
            nc.vector.scalar_tensor_tensor(
                res[:], negs[:], -2.0, sall[:],
                mybir.AluOpType.mult, mybir.AluOpType.add,
            )
            nc.sync.dma_start(out_d.rearrange("(p i) -> p i", i=N_SLOTS), res[:])
    nc.compile()
    return nc


def _prep_host(locations3d, anchor_locations3d, anchor_coeffs,
               anchor_parameters):
    """Tile the points, select per-tile anchors, build factor matrices."""
    x64 = locations3d.astype(np.float64)            # [N, 3]
    a64 = anchor_locations3d.astype(np.float64)     # [M, 3]
    c64 = anchor_coeffs.astype(np.float64)          # [M]
    p64 = anchor_parameters.astype(np.float64)      # [M]

    w = 0.5 / (p64 * p64)                           # [M]
    a_sq = (a64 * a64).sum(axis=1)                  # [M]
    x_sq = (x64 * x64).sum(axis=1)                  # [N]
    ln_c = np.log(np.maximum(np.abs(c64), 1e-300))
    ln_c = np.maximum(ln_c, -60.0)
    neg_mask = c64 < 0

    # anchor-side factor rows [14, M]: per product (vh, vm, vh), then
    # const (ch, cm); point-side rows built to match (uh, uh, um / 1, 1).
    anchor_factors = [2.0 * w * a64[:, 0], 2.0 * w * a64[:, 1],
                      2.0 * w * a64[:, 2], -w]
    point_factors = [x64[:, 0], x64[:, 1], x64[:, 2], x_sq]
    r_rows, p_rows = [], []
    for u, v in zip(point_factors, anchor_factors):
        uh, um = _split2(u)
        vh, vm = _split2(v)
        p_rows.extend([uh, uh, um])
        r_rows.extend([vh, vm, vh])
    ch, cm = _split2(-w * a_sq + ln_c)
    ones = np.ones(x_sq.shape[0], dtype=_BF16)
    p_rows.extend([ones, ones])
    r_rows.extend([ch, cm])
    P14 = np.stack(p_rows).astype(_BF16)            # [14, N]
    R14 = np.stack(r_rows).astype(_BF16)            # [14, M]

    # dummy (padding) column: arg = -60 -> exp ~ 0
    pad_col = np.zeros((K_ROWS, 1), dtype=_BF16)
    pad_col[K_ROWS - 2, 0] = _BF16(-60.0)

    # spatial tiling + per-tile anchor selection
    tiles = _median_split_tiles(x64)                # [1024, 128]
    tc_ = x64[tiles].mean(axis=1)                   # [1024, 3]
    trad = np.linalg.norm(x64[tiles] - tc_[:, None, :], axis=2).max(axis=1)
    D = np.linalg.norm(tc_[:, None, :] - a64[None, :, :], axis=2)
    gap = np.maximum(D - trad[:, None], 0.0)
    sig = w[None, :] * gap * gap - ln_c[None, :] < THRESH  # [1024, M]
    counts = sig.sum(axis=1)

    # sort tiles by cost desc; consecutive groups of 8 -> one per core
    order_t = np.argsort(-counts, kind="stable")
    n_tiles = tiles.shape[0]
    assert n_tiles == N_SLOTS * N_CORES

    s_slot = np.zeros(N_SLOTS, dtype=np.int64)
    s_neg_slot = np.zeros(N_SLOTS, dtype=np.int64)
    tile_cols = [[None] * N_SLOTS for _ in range(N_CORES)]
    gids = [np.zeros((P, N_SLOTS), dtype=np.int64) for _ in range(N_CORES)]
    for k in range(N_SLOTS):
        grp = order_t[k * N_CORES : (k + 1) * N_CORES]
        negc = np.zeros(N_CORES, dtype=np.int64)
        cols = []
        for c, t in enumerate(grp):
            s = np.where(sig[t])[0]
            sn = s[neg_mask[s]]
            sp = s[~neg_mask[s]]
            cols.append((sn, sp))
            negc[c] = len(sn)
            gids[c][:, k] = tiles[t]
        sn_max = -(-max(int(negc.max()), 4) // 4) * 4
        sp_max = max(len(b) for _, b in cols)
        s_max = -(-(sn_max + max(sp_max, 4)) // 8) * 8
        s_slot[k] = s_max
        s_neg_slot[k] = sn_max
        for c, (sn, sp) in enumerate(cols):
            tile_cols[c][k] = (sn, sp, sn_max, s_max)
    rt_total = int(s_slot.sum())

    # build per-core rt [14, rt_total] and pm [14, N_LOC]
    rt_cores, pm_cores = [], []
    for c in range(N_CORES):
        segs = []
        for k in range(N_SLOTS):
            sn, sp, sn_max, s_max = tile_cols[c][k]
            seg = np.empty((K_ROWS, s_max), dtype=_BF16)
            npad_n = sn_max - len(sn)
            npad_t = s_max - sn_max - len(sp)
            seg[:, : len(sn)] = R14[:, sn]
            seg[:, len(sn) : sn_max] = pad_col
            seg[:, sn_max : sn_max + len(sp)] = R14[:, sp]
            seg[:, sn_max + len(sp) :] = pad_col
            segs.append(seg)
        rt_cores.append(np.ascontiguousarray(np.concatenate(segs, axis=1)))
        pm = np.empty((K_ROWS, N_LOC), dtype=_BF16)
        for k in range(N_SLOTS):
            pm[:, P * k : P * (k + 1)] = P14[:, gids[c][:, k]]
        pm_cores.append(pm)

    return pm_cores, rt_cores, gids, s_slot, s_neg_slot, rt_total


def kernel(locations3d, anchor_locations3d, anchor_coeffs, anchor_parameters):
    assert locations3d.shape == (N_POINTS, 3)
    assert anchor_locations3d.shape == (N_ANCH, 3)

    pm_cores, rt_cores, gids, s_slot, s_neg_slot, rt_total = _prep_host(
        locations3d, anchor_locations3d, anchor_coeffs, anchor_parameters
    )

    key = (tuple(s_slot), tuple(s_neg_slot))
    nc = _program_cache.get(key)
    if nc is None:
        nc = _build_program(s_slot, s_neg_slot, rt_total)
        _program_cache[key] = nc

    in_maps = [
        {"pm": pm_cores[c], "rt": rt_cores[c]} for c in range(N_CORES)
    ]
    res = run_bass_kernel_spmd(
        nc, in_maps, core_ids=list(range(N_CORES)), trace=TRACE
    )
    global LAST_RESULTS
    LAST_RESULTS = res
    out = np.empty(N_POINTS, dtype=np.float32)
    for c in range(N_CORES):
        out[gids[c].reshape(-1)] = res.results[c]["out"]
    return out


# revision 13
# speedup vs baseline: 2.1590x; 1.2103x over previous
"""Trainium2 Bass kernel for LinearPotential (RBF potential evaluation).

out[n] = sum_m c_m * exp(-||x_n - a_m||^2 * w_m),  w_m = 0.5 / p_m^2

Strategy: the ScalarE exp ACTIVATE (1 elem/cycle/lane @ 1.2 GHz) is the hard
bottleneck for the dense [N, M] evaluation, so the kernel drops pairs that
cannot contribute: most anchors are narrow (w up to ~50) and their Gaussian
reaches only a small neighborhood.

  - Host: recursively median-split the points into 1024 spatially tight
    tiles of 128.  For each tile keep only anchors with
    w*(max(0, |center-a|-radius))^2 - ln|c| < T  (a conservative bound on
    the best-case exponent for any point in the tile; dropped terms are
    each < e^-T).  Mean kept ~= 48% of anchors.
  - The 8 cores run ONE compiled program (SPMD), so per-slot trip counts
    are shared: tiles are sorted by kept-anchor count and dealt to cores in
    groups of 8 consecutive tiles -> per-slot padding waste is tiny.
  - Device per slot: matmul (K=14 bf16 factor rows, split products for
    ~2^-17 arg accuracy) -> PSUM [128, S] -> ScalarE Exp with free-dim
    accumulation (S_all) + fp16 elementwise scratch -> VectorE re-sum of
    the negative-coefficient prefix (S_neg) -> out = S_all - 2*S_neg.

Self-contained: hardcodes shapes for N=131072 points, M=2048 anchors.
"""

import numpy as np
import ml_dtypes

import concourse.tile as tile
from concourse import bacc, mybir
from concourse.bass_utils import run_bass_kernel_spmd

N_CORES = 8
N_POINTS = 131072
N_ANCH = 2048
N_LOC = N_POINTS // N_CORES  # 16384 points per core
P = 128                      # partition dim / points per tile
N_SLOTS = N_LOC // P         # 128 program slots per core
K_ROWS = 14                  # 4 products x 3 split rows + 2 const rows
MM_N = 512                   # matmul free-dim tile (one PSUM bank, fp32)
THRESH = 5.0                 # keep anchors with w*mindist^2 - ln|c| < THRESH
DMA_GROUP = 4                # slots per rt DMA

_BF16 = ml_dtypes.bfloat16

_program_cache: dict = {}

# test-harness hooks (no effect on grading: default off)
TRACE = False
LAST_RESULTS = None


def _split2(v: np.ndarray):
    """Split fp64 array into 2 bf16 components h+m ~ v (rel err ~2^-17)."""
    h = v.astype(_BF16)
    m = (v - h.astype(np.float64)).astype(_BF16)
    return h, m


def _median_split_tiles(x: np.ndarray):
    """Recursively split N points into N/128 tiles of exactly 128 points
    by median cuts along the widest dimension. Returns [n_tiles, 128]
    int64 index array."""
    n = x.shape[0]
    idx = np.arange(n)
    groups = [idx]
    while groups[0].shape[0] > P:
        nxt = []
        for g in groups:
            pts = x[g]
            dim = int(np.argmax(pts.max(0) - pts.min(0)))
            half = g.shape[0] // 2
            part = np.argpartition(pts[:, dim], half)
            nxt.append(g[part[:half]])
            nxt.append(g[part[half:]])
        groups = nxt
    return np.stack(groups)


def _build_program(s_slot, s_neg_slot, k_acc, rt_total):
    """Build + compile the per-core Bass program (same on all 8 cores)."""
    nc = bacc.Bacc("TRN2", target_bir_lowering=False, debug=False,
                   num_devices=N_CORES)
    pm_d = nc.dram_tensor("pm", [K_ROWS, N_LOC], mybir.dt.bfloat16,
                          kind="ExternalInput").ap()
    rt_d = nc.dram_tensor("rt", [K_ROWS, rt_total], mybir.dt.bfloat16,
                          kind="ExternalInput").ap()
    out_d = nc.dram_tensor("out", [N_LOC], mybir.dt.float32,
                           kind="ExternalOutput").ap()

    rt_off = np.concatenate([[0], np.cumsum(s_slot)])
    exp_f = mybir.ActivationFunctionType.Exp
    with tile.TileContext(nc) as tc:
        with (
            tc.tile_pool(name="const", bufs=1) as cpool,
            tc.tile_pool(name="rtp", bufs=3) as rtpool,
            tc.tile_pool(name="scp", bufs=3) as spool,
            tc.tile_pool(name="psum", bufs=2, space="PSUM") as ppool,
        ):
            pm = cpool.tile([K_ROWS, N_LOC], mybir.dt.bfloat16)
            sall = cpool.tile([P, N_SLOTS], mybir.dt.float32)
            negs = cpool.tile([P, N_SLOTS], mybir.dt.float32)
            res = cpool.tile([P, N_SLOTS], mybir.dt.float32)

            # interleave point-matrix chunk loads with rt group loads so
            # the first matmuls start early (all on the same SP queue)
            n_chunks = 16
            cw = N_LOC // n_chunks
            rt_bufs = {}
            for g in range(0, N_SLOTS, DMA_GROUP):
                c = g // DMA_GROUP
                if c < n_chunks:
                    nc.sync.dma_start(
                        pm[:, c * cw : (c + 1) * cw],
                        pm_d[:, c * cw : (c + 1) * cw],
                    )
                lo = int(rt_off[g])
                hi = int(rt_off[min(g + DMA_GROUP, N_SLOTS)])
                rt = rtpool.tile([K_ROWS, hi - lo], mybir.dt.bfloat16)
                nc.sync.dma_start(rt[:], rt_d[:, lo:hi])
                for k in range(g, min(g + DMA_GROUP, N_SLOTS)):
                    rt_bufs[k] = (rt, int(rt_off[k]) - lo)

            for k in range(N_SLOTS):
                S = int(s_slot[k])
                Sn = int(s_neg_slot[k])
                rt, base = rt_bufs[k]
                ps = ppool.tile([P, 2048], mybir.dt.float32)
                lhsT = pm[:, P * k : P * (k + 1)]
                for j in range(0, S, MM_N):
                    L = min(MM_N, S - j)
                    nc.tensor.matmul(
                        ps[:, j : j + L],
                        lhsT=lhsT,
                        rhs=rt[:, base + j : base + j + L],
                        start=True,
                        stop=True,
                    )
                # Exp over the kept anchors, elementwise into fp16 scratch.
                # accum-mode slots (k < k_acc): ScalarE's accumulator gives
                # S_all; VectorE re-sums the neg window -> S_all - 2*S_neg.
                # dve-mode slots: ScalarE skips the (285ns) accumulator
                # drain; VectorE sums both windows -> S_pos - S_neg.
                sc = spool.tile([P, 2048], mybir.dt.float16)
                if k < k_acc:
                    nc.scalar.activation(
                        sc[:, :S], ps[:, :S], exp_f,
                        accum_out=sall[:, k : k + 1],
                    )
                else:
                    nc.scalar.activation(sc[:, :S], ps[:, :S], exp_f)
                    nc.vector.reduce_sum(
                        sall[:, k : k + 1], sc[:, Sn:S],
                        axis=mybir.AxisListType.X,
                    )
                nc.vector.reduce_sum(
                    negs[:, k : k + 1], sc[:, :Sn], axis=mybir.AxisListType.X
                )
            if k_acc > 0:
                nc.vector.scalar_tensor_tensor(
                    res[:, :k_acc], negs[:, :k_acc], -2.0, sall[:, :k_acc],
                    mybir.AluOpType.mult, mybir.AluOpType.add,
                )
            if k_acc < N_SLOTS:
                nc.vector.scalar_tensor_tensor(
                    res[:, k_acc:], negs[:, k_acc:], -1.0, sall[:, k_acc:],
                    mybir.AluOpType.mult, mybir.AluOpType.add,
                )
            nc.sync.dma_start(out_d.rearrange("(p i) -> p i", i=N_SLOTS), res[:])
    nc.compile()
    return nc


def _prep_host(locations3d, anchor_locations3d, anchor_coeffs,
               anchor_parameters):
    """Tile the points, select per-tile anchors, build factor matrices."""
    x64 = locations3d.astype(np.float64)            # [N, 3]
    a64 = anchor_locations3d.astype(np.float64)     # [M, 3]
    c64 = anchor_coeffs.astype(np.float64)          # [M]
    p64 = anchor_parameters.astype(np.float64)      # [M]

    w = 0.5 / (p64 * p64)                           # [M]
    a_sq = (a64 * a64).sum(axis=1)                  # [M]
    x_sq = (x64 * x64).sum(axis=1)                  # [N]
    ln_c = np.log(np.maximum(np.abs(c64), 1e-300))
    ln_c = np.maximum(ln_c, -60.0)
    neg_mask = c64 < 0

    # anchor-side factor rows [14, M]: per product (vh, vm, vh), then
    # const (ch, cm); point-side rows built to match (uh, uh, um / 1, 1).
    anchor_factors = [2.0 * w * a64[:, 0], 2.0 * w * a64[:, 1],
                      2.0 * w * a64[:, 2], -w]
    point_factors = [x64[:, 0], x64[:, 1], x64[:, 2], x_sq]
    r_rows, p_rows = [], []
    for u, v in zip(point_factors, anchor_factors):
        uh, um = _split2(u)
        vh, vm = _split2(v)
        p_rows.extend([uh, uh, um])
        r_rows.extend([vh, vm, vh])
    ch, cm = _split2(-w * a_sq + ln_c)
    ones = np.ones(x_sq.shape[0], dtype=_BF16)
    p_rows.extend([ones, ones])
    r_rows.extend([ch, cm])
    P14 = np.stack(p_rows).astype(_BF16)            # [14, N]
    R14 = np.stack(r_rows).astype(_BF16)            # [14, M]

    # dummy (padding) column: arg = -60 -> exp ~ 0
    pad_col = np.zeros((K_ROWS, 1), dtype=_BF16)
    pad_col[K_ROWS - 2, 0] = _BF16(-60.0)

    # spatial tiling + per-tile anchor selection: cheap center+radius
    # bound first, then the exact min distance over the tile's points
    tiles = _median_split_tiles(x64)                # [1024, 128]
    tc_ = x64[tiles].mean(axis=1)                   # [1024, 3]
    trad = np.linalg.norm(x64[tiles] - tc_[:, None, :], axis=2).max(axis=1)
    D = np.linalg.norm(tc_[:, None, :] - a64[None, :, :], axis=2)
    gap = np.maximum(D - trad[:, None], 0.0)
    sig_bound = w[None, :] * gap * gap - ln_c[None, :] < THRESH  # [1024, M]
    sig_lists = []
    for t in range(tiles.shape[0]):
        cand = np.where(sig_bound[t])[0]
        pts = x64[tiles[t]]
        d2min = ((pts[:, None, :] - a64[cand][None, :, :]) ** 2).sum(-1).min(0)
        sig_lists.append(cand[w[cand] * d2min - ln_c[cand] < THRESH])
    counts = np.array([len(s) for s in sig_lists])

    # group tiles into slots of 8 (one per core): primary sort by total
    # count desc, then within blocks of 32 re-sort by neg count so each
    # group of 8 has similar (neg, pos) splits -> minimal window padding
    sn_list = np.array([int(neg_mask[s].sum()) for s in sig_lists])
    sp_list = counts - sn_list
    order_t = np.argsort(-counts, kind="stable")
    for b in range(0, len(order_t), 32):
        blk = order_t[b : b + 32]
        order_t[b : b + 32] = blk[np.argsort(-sn_list[blk], kind="stable")]
    n_tiles = tiles.shape[0]
    assert n_tiles == N_SLOTS * N_CORES

    s_slot = np.zeros(N_SLOTS, dtype=np.int64)
    s_neg_slot = np.zeros(N_SLOTS, dtype=np.int64)
    tile_cols = [[None] * N_SLOTS for _ in range(N_CORES)]
    gids_all = np.zeros((N_CORES, P, N_SLOTS), dtype=np.int64)
    for k in range(N_SLOTS):
        grp = order_t[k * N_CORES : (k + 1) * N_CORES]
        cols = []
        for c, t in enumerate(grp):
            s = sig_lists[t]
            cols.append((s[neg_mask[s]], s[~neg_mask[s]]))
            gids_all[c, :, k] = tiles[t]
        # layout per core: [neg | pad to sn_max | pos | pad to s_max];
        # the neg re-sum window [0, sn_max) then holds only negs + zeros
        sn_max = max(max(len(a) for a, _ in cols), 4)
        s_max = -(-(sn_max + max(len(b) for _, b in cols)) // 8) * 8
        s_slot[k] = s_max
        s_neg_slot[k] = sn_max
        for c, (sn, sp) in enumerate(cols):
            tile_cols[c][k] = (sn, sp, sn_max, s_max)

    # hybrid summation: for slots flagged use_dve, ScalarE skips the
    # accumulator (saving ~285ns/slot) and VectorE sums both windows.
    # Greedily pick modes to balance ScalarE vs VectorE busy time.
    sc_load = dve_load = 0.0
    use_dve = np.zeros(N_SLOTS, dtype=bool)
    for k in np.argsort(-s_slot, kind="stable"):
        S, Sn = float(s_slot[k]), float(s_neg_slot[k])
        act = (S + 86) / 1.2
        d_neg = (58 + Sn) / 0.96
        d_pos = (58 + S - Sn) / 0.96
        max_a = max(sc_load + act + 285, dve_load + d_neg)
        max_d = max(sc_load + act, dve_load + d_neg + d_pos)
        if max_d < max_a:
            use_dve[k] = True
            sc_load += act
            dve_load += d_neg + d_pos
        else:
            sc_load += act + 285
            dve_load += d_neg
    # relabel slots so accum-mode slots occupy columns [0, kA)
    perm = np.concatenate([np.where(~use_dve)[0], np.where(use_dve)[0]])
    k_acc = int((~use_dve).sum())
    s_slot = s_slot[perm]
    s_neg_slot = s_neg_slot[perm]
    gids_all = gids_all[:, :, perm]
    tile_cols = [[tile_cols[c][k] for k in perm] for c in range(N_CORES)]
    rt_total = int(s_slot.sum())

    # build per-core rt [14, rt_total] and pm [14, N_LOC]
    rt_cores, pm_cores = [], []
    for c in range(N_CORES):
        segs = []
        for k in range(N_SLOTS):
            sn, sp, sn_max, s_max = tile_cols[c][k]
            seg = np.empty((K_ROWS, s_max), dtype=_BF16)
            seg[:, : len(sn)] = R14[:, sn]
            seg[:, len(sn) : sn_max] = pad_col
            seg[:, sn_max : sn_max + len(sp)] = R14[:, sp]
            seg[:, sn_max + len(sp) :] = pad_col
            segs.append(seg)
        rt_cores.append(np.ascontiguousarray(np.concatenate(segs, axis=1)))
        pm = np.empty((K_ROWS, N_LOC), dtype=_BF16)
        for k in range(N_SLOTS):
            pm[:, P * k : P * (k + 1)] = P14[:, gids_all[c, :, k]]
        pm_cores.append(pm)

    gids = [gids_all[c] for c in range(N_CORES)]
    return pm_cores, rt_cores, gids, s_slot, s_neg_slot, k_acc, rt_total


def kernel(locations3d, anchor_locations3d, anchor_coeffs, anchor_parameters):
    assert locations3d.shape == (N_POINTS, 3)
    assert anchor_locations3d.shape == (N_ANCH, 3)

    pm_cores, rt_cores, gids, s_slot, s_neg_slot, k_acc, rt_total = _prep_host(
        locations3d, anchor_locations3d, anchor_coeffs, anchor_parameters
    )

    key = (tuple(s_slot), tuple(s_neg_slot), k_acc)
    nc = _program_cache.get(key)
    if nc is None:
        nc = _build_program(s_slot, s_neg_slot, k_acc, rt_total)
        _program_cache[key] = nc

    in_maps = [
        {"pm": pm_cores[c], "rt": rt_cores[c]} for c in range(N_CORES)
    ]
    res = run_bass_kernel_spmd(
        nc, in_maps, core_ids=list(range(N_CORES)), trace=TRACE
    )
    global LAST_RESULTS
    LAST_RESULTS = res
    out = np.empty(N_POINTS, dtype=np.float32)
    for c in range(N_CORES):
        out[gids[c].reshape(-1)] = res.results[c]["out"]
    return out
